# revision 8
# baseline (speedup 1.0000x reference)
"""Trainium2 Bass kernel for a 16-head dense attention block (B=1, S=2048, D=2048).

Sharding: 2 heads per core across 8 cores (tensor parallel on heads).
The reference's (deliberate) transpose(2,3)+reshape before the output
projection makes output rows [h*128:(h+1)*128) depend ONLY on head h, so
per-core outputs are disjoint row blocks -> host-side concat, no collectives.

Fully fused single-loop pipeline; per iteration g (q-group of 512):
  B: scores S^T[k,q] = K @ Q^T (f32r) for all active k-tiles, exp on ScalarE
     -> bf16 prob tiles (kept live for the whole group), PE-paced by weaving
     in A's matmuls.
  A: QKV projections for group g+1 (all-bf16 matmuls, full rate), RoPE on
     DVE with pair-swap via SBUF->SBUF DMA; V stored bf16.
  C: O[q,dh] computed directly per 128-q chunk (stationary = prob chunk,
     moving = V block, bf16): accumulation groups run j-outer/kt-inner so
     each PSUM bank hosts one contiguous group at a time.  Row-sums are
     near-free flipped matmuls (moving = ones [k,1], 1 row).  Epilogue:
     DVE reciprocal + 4 tensor_scalar_muls fold 1/rowsum into bf16 O.
  D: output projection partials R_h += O_h(g)^T @ wo^T(g-rows) streamed
     per (mg,h) through one PSUM bank ring, ScalarE PSUM->SBUF copy, and
     DMA-accumulated into DRAM out (gpsimd software-DGE adds).
Causal-mask block skipping at 128x128 chunk granularity (exp underflow).
PSUM: pq(2) + pv/sums(1) + sc(3) + pso/psr(2) = 8 banks exactly.
"""

import math

import numpy as np

S = 2048
D = 2048
H = 16
DH = 128
N_CORES = 8
HPC = H // N_CORES          # heads per core
NH = HPC * DH               # per-core head rows (256)
P = 128
QG = 512                    # q/s-group width
NQG = S // QG               # 4
NKT = S // P                # 16 k tiles
NDT = D // P                # 16 d tiles
NCH = QG // P               # 4 chunks per group

SKIP, NOMASK, MASKED = 0, 1, 2

_CACHE = {}


def _build(block_kind):
    """block_kind: tuple of NQG tuples of NKT (kind, first_chunk) pairs."""
    import os
    import concourse.tile as tile
    from concourse import bacc, mybir
    from concourse.alu_op_type import AluOpType

    B = lambda k, d: int(os.environ.get(k, d))
    f32 = mybir.dt.float32
    f32r = mybir.dt.float32r
    bf16 = mybir.dt.bfloat16
    EXP = mybir.ActivationFunctionType.Exp

    nc = bacc.Bacc("TRN2", target_bir_lowering=False, debug=False,
                   num_devices=N_CORES)

    xT = nc.dram_tensor("xT", [D, S], bf16, kind="ExternalInput").ap()
    wqT = nc.dram_tensor("wqT", [D, NH], bf16, kind="ExternalInput").ap()
    wkT = nc.dram_tensor("wkT", [D, NH], bf16, kind="ExternalInput").ap()
    wvT = nc.dram_tensor("wvT", [D, NH], bf16, kind="ExternalInput").ap()
    maskT = nc.dram_tensor("maskT", [S, S], bf16, kind="ExternalInput").ap()
    woT = nc.dram_tensor("woT", [S, D], bf16, kind="ExternalInput").ap()
    cq = nc.dram_tensor("cq", [DH, S], f32, kind="ExternalInput").ap()
    sq = nc.dram_tensor("sq", [DH, S], f32, kind="ExternalInput").ap()
    ck = nc.dram_tensor("ck", [DH, S], f32, kind="ExternalInput").ap()
    sk = nc.dram_tensor("sk", [DH, S], f32, kind="ExternalInput").ap()
    ones = nc.dram_tensor("ones", [P, 1], bf16, kind="ExternalInput").ap()
    out = nc.dram_tensor("out", [NH, D], f32, kind="ExternalOutput").ap()

    xT_v = xT.rearrange("(t p) s -> t p s", p=P)           # [16,128,S]
    wT_v = {"q": wqT.rearrange("(t p) n -> t p n", p=P),
            "k": wkT.rearrange("(t p) n -> t p n", p=P),
            "v": wvT.rearrange("(t p) n -> t p n", p=P)}
    maskT_v = maskT.rearrange("(t p) s -> t p s", p=P)
    woT_v = woT.rearrange("(t p) m -> t p m", p=P)

    # per-group mask info
    act = []          # active kts per g
    jkt = []          # per g: {j: [kts writing chunk j]}
    for g in range(NQG):
        kinds = block_kind[g]
        a = [kt for kt in range(NKT) if kinds[kt][0] != SKIP]
        act.append(a)
        jkt.append({j: [kt for kt in a if kinds[kt][1] <= j]
                    for j in range(NCH)})

    with tile.TileContext(nc) as tc:
        with tc.tile_pool(name="pers", bufs=1) as pers, \
             tc.tile_pool(name="wsb", bufs=1) as wsb, \
             tc.tile_pool(name="xssb", bufs=1) as xssb, \
             tc.tile_pool(name="ptsb", bufs=1) as ptsb, \
             tc.tile_pool(name="attsb", bufs=1) as attsb, \
             tc.tile_pool(name="wosb", bufs=B("BW", 16)) as wosb, \
             tc.tile_pool(name="rssb", bufs=2) as rssb, \
             tc.tile_pool(name="ropesb", bufs=B("BR", 2)) as ropesb, \
             tc.tile_pool(name="ps_q", bufs=B("BQ", 2), space="PSUM") as ps_q, \
             tc.tile_pool(name="ps_vs", bufs=1, space="PSUM") as ps_vs, \
             tc.tile_pool(name="ps_sc", bufs=B("BS", 3), space="PSUM") as ps_sc, \
             tc.tile_pool(name="ps_or", bufs=B("BO", 2), space="PSUM") as ps_or:

            qt = [pers.tile([P, S], f32, tag=f"qt{h}", name=f"qt{h}")
                  for h in range(HPC)]
            kt_ = [pers.tile([P, S], f32, tag=f"kt{h}", name=f"kt{h}")
                   for h in range(HPC)]
            vt = pers.tile([P, NKT, NH], bf16, tag="v")
            o_sb = [pers.tile([P, S], bf16, tag=f"ot{h}", name=f"ot{h}")
                    for h in range(HPC)]
            rt = [pers.tile([P, NKT], f32, tag=f"rt{h}", name=f"rt{h}")
                  for h in range(HPC)]

            # weights + rope constants (scalar queue); chunk-major order so
            # the first q-projection matmul's operands land first
            wts = {kind: wsb.tile([P, NDT, NH], bf16, tag=f"w{kind}",
                                  name=f"w{kind}")
                   for kind in ("q", "k", "v")}
            for c in range(4):
                cs = slice(c * 4, c * 4 + 4)
                for kind in ("q", "k", "v"):
                    nc.scalar.dma_start(
                        wts[kind][:, cs],
                        wT_v[kind][cs].rearrange("t p n -> p t n"))
            ones_t = pers.tile([P, 1], bf16, tag="ones")
            nc.scalar.dma_start(ones_t[:], ones[:])
            rope_t = {}
            for nm, src in (("cq", cq), ("sq", sq), ("ck", ck), ("sk", sk)):
                t = wsb.tile([DH, S], f32, tag=nm, name=nm)
                nc.scalar.dma_start(t[:], src[:])
                rope_t[nm] = t

            xs_tiles = {}     # g -> list of 4 chunk tiles

            def issue_xs(g):
                xs_c = []
                sl = slice(g * QG, (g + 1) * QG)
                for c in range(4):
                    cs = slice(c * 4, c * 4 + 4)
                    xc = xssb.tile([P, 4, QG], bf16, tag=f"xs{c}", bufs=2,
                                   name=f"xs{c}")
                    nc.sync.dma_start(
                        xc[:], xT_v[cs, :, sl].rearrange("t p s -> p t s"))
                    xs_c.append(xc)
                xs_tiles[g] = xs_c

            def qkv_items(g):
                """Flat list of emit-callbacks for QKV projections of group
                g: matmuls at item granularity; group-final items also emit
                the PSUM->SBUF copy (+ rope)."""
                xs_c = xs_tiles[g]
                sl = slice(g * QG, (g + 1) * QG)
                items = []

                def xs_ap(dt, *rest):
                    return xs_c[dt // 4][(slice(None), dt % 4) + rest]

                def emit_rope(kind, h, ps):
                    m = qt[h] if kind == "q" else kt_[h]
                    cn, sn = ("cq", "sq") if kind == "q" else ("ck", "sk")
                    nc.vector.tensor_copy(m[:, sl].bitcast(f32r), ps[:])
                    sw = ropesb.tile([P, QG], f32, tag="sw", name="sw")
                    m_v = m[:, sl].rearrange("(j b) s -> j b s", b=2)
                    sw_v = sw.rearrange("(j b) s -> j b s", b=2)
                    nc.sync.dma_start(sw_v[:, 0], m_v[:, 1])
                    nc.sync.dma_start(sw_v[:, 1], m_v[:, 0])
                    t1 = ropesb.tile([P, QG], f32, tag="t1", name="t1")
                    nc.vector.tensor_mul(t1[:], m[:, sl], rope_t[cn][:, sl])
                    nc.vector.tensor_mul(sw[:], sw[:], rope_t[sn][:, sl])
                    nc.vector.tensor_add(m[:, sl].bitcast(f32r), t1[:], sw[:])

                for kind in ("q", "k"):
                    for h in range(HPC):
                        ps = ps_q.tile([P, QG], f32, tag="pq", name="pq")
                        for dt in range(NDT):
                            def mk(kind=kind, h=h, ps=ps, dt=dt):
                                nc.tensor.matmul(
                                    ps[:], wts[kind][:, dt, h * P:(h + 1) * P],
                                    xs_ap(dt), start=(dt == 0),
                                    stop=(dt == NDT - 1))
                                if dt == NDT - 1:
                                    emit_rope(kind, h, ps)
                            items.append(mk)
                for st in range(g * 4, g * 4 + 4):
                    ps = ps_vs.tile([P, QG], f32, tag="vs", name="pvs")
                    lsl = slice((st % 4) * P, (st % 4) * P + P)
                    for dt in range(NDT):
                        def mk(st=st, ps=ps, lsl=lsl, dt=dt):
                            nc.tensor.matmul(ps[:, 0:NH], xs_ap(dt, lsl),
                                             wts["v"][:, dt], start=(dt == 0),
                                             stop=(dt == NDT - 1))
                            if dt == NDT - 1:
                                nc.vector.tensor_copy(vt[:, st], ps[:, 0:NH])
                        items.append(mk)
                return items

            # prologue: x for groups 0 and 1; QKV(0) unweaved
            issue_xs(0)
            issue_xs(1)
            for it in qkv_items(0):
                it()

            pts = {}          # (g, h, kt) -> pt tile

            def emit_mask_dma(g, kt):
                kind, fc = block_kind[g][kt]
                x_off = fc * P
                mt = attsb.tile([P, QG], bf16, tag="mt", bufs=4, name="mt")
                nc.sync.dma_start(
                    mt[:, x_off:QG],
                    maskT_v[kt][:, g * QG + x_off:(g + 1) * QG])
                return mt

            def emit_b(g, kt, mt):
                kind, fc = block_kind[g][kt]
                sc_off = min(fc * P, QG - 2 * P)
                x_off = fc * P
                osl_sc = slice(sc_off, QG)
                qsl_sc = slice(g * QG + sc_off, (g + 1) * QG)
                osl_x = slice(x_off, QG)
                ksl = slice(kt * P, (kt + 1) * P)
                for h in range(HPC):
                    ps = ps_sc.tile([P, QG], f32, tag="sc", name="sc")
                    nc.tensor.matmul(ps[:, osl_sc],
                                     kt_[h][:, ksl].bitcast(f32r),
                                     qt[h][:, qsl_sc].bitcast(f32r),
                                     start=True, stop=True)
                    pt = ptsb.tile([P, QG], bf16, tag=f"pt{h}", bufs=NKT,
                                   name=f"pt{h}")
                    if kind == MASKED:
                        sm = attsb.tile([P, QG], f32, tag="sm", bufs=2,
                                        name="sm")
                        nc.vector.tensor_add(sm[:, osl_x], ps[:, osl_x],
                                             mt[:, osl_x])
                        nc.scalar.activation(pt[:, osl_x], sm[:, osl_x], EXP)
                    else:
                        nc.scalar.activation(pt[:, osl_x], ps[:, osl_x], EXP)
                    pts[g, h, kt] = pt

            # number of g+1 k-tiles whose scores/exp are emitted early, at
            # the end of iteration g (fills ScalarE while iteration g+1 has
            # no QKV filler left)
            EARLY = {3: B("BE", 8)}

            for g in range(NQG):
                kinds = block_kind[g]
                if g + 2 < NQG:
                    issue_xs(g + 2)
                early = EARLY.get(g, 0)
                main_kts = act[g][early:]
                mts = {kt: emit_mask_dma(g, kt) for kt in main_kts
                       if kinds[kt][0] == MASKED}

                # ---- B (scores+exp) weaved with A (QKV g+1) ----
                a_items = qkv_items(g + 1) if g + 1 < NQG else []
                n_b = max(len(main_kts), 1)
                emitted = 0
                for i, kt in enumerate(main_kts):
                    emit_b(g, kt, mts.get(kt))
                    want = len(a_items) * (i + 1) // n_b
                    while emitted < want:
                        a_items[emitted]()
                        emitted += 1
                while emitted < len(a_items):
                    a_items[emitted]()
                    emitted += 1

                # ---- C: O + rowsums, j-outer/kt-inner groups ----
                pss = ps_vs.tile([P, QG], f32, tag="vs", name="pss")
                pso = {}
                for h in range(HPC):
                    pso[h] = ps_or.tile([P, QG], f32, tag="or", name="pso")
                    for j in range(NCH):
                        jsl_l = slice(j * P, (j + 1) * P)
                        ks = jkt[g][j]
                        for kt in ks:
                            nc.tensor.matmul(
                                pso[h][:, jsl_l], pts[g, h, kt][:, jsl_l],
                                vt[:, kt, h * P:(h + 1) * P],
                                start=(kt == ks[0]), stop=(kt == ks[-1]))
                        col = NH + h * NCH + j
                        for kt in ks:
                            nc.tensor.matmul(
                                pss[:, col:col + 1], pts[g, h, kt][:, jsl_l],
                                ones_t[:],
                                start=(kt == ks[0]), stop=(kt == ks[-1]))
                    c0 = NH + h * NCH
                    nc.vector.reciprocal(rt[h][:, g * NCH:(g + 1) * NCH],
                                         pss[:, c0:c0 + NCH])
                    for j in range(NCH):
                        jt = g * NCH + j
                        jsl = slice(jt * P, (jt + 1) * P)
                        nc.vector.tensor_scalar_mul(
                            o_sb[h][:, jsl], pso[h][:, j * P:(j + 1) * P],
                            rt[h][:, jt:jt + 1])

                # ---- D: output projection partials for this group ----
                for mg in range(NQG):
                    msl = slice(mg * QG, (mg + 1) * QG)
                    wt_c = []
                    for ji in range(NCH):
                        jt = g * NCH + ji
                        wt = wosb.tile([P, QG], bf16, tag="wo", name="wt")
                        nc.scalar.dma_start(wt[:], woT_v[jt][:, msl])
                        wt_c.append(wt)
                    for h in range(HPC):
                        psr = ps_or.tile([P, QG], f32, tag="or", name="psr")
                        for ji in range(NCH):
                            jt = g * NCH + ji
                            jsl = slice(jt * P, (jt + 1) * P)
                            nc.tensor.matmul(psr[:], o_sb[h][:, jsl],
                                             wt_c[ji], start=(ji == 0),
                                             stop=(ji == NCH - 1))
                        rs = rssb.tile([P, QG], f32, tag="rs", name="rs")
                        nc.vector.tensor_copy(rs[:], psr[:])
                        if g == 0:
                            nc.sync.dma_start(out[h * P:(h + 1) * P, msl],
                                              rs[:])
                        else:
                            nc.gpsimd.dma_start(out[h * P:(h + 1) * P, msl],
                                                rs[:],
                                                accum_op=AluOpType.add)

                # ---- early scores/exp for the next group ----
                nearly = EARLY.get(g + 1, 0)
                if nearly:
                    early_kts = act[g + 1][:nearly]
                    emts = {kt: emit_mask_dma(g + 1, kt) for kt in early_kts
                            if block_kind[g + 1][kt][0] == MASKED}
                    for kt in early_kts:
                        emit_b(g + 1, kt, emts.get(kt))

    nc.compile()
    return nc


def _classify_mask(maskT):
    """Per (g, kt) block: (kind, first_chunk).  kind is SKIP if exp(s+m)
    underflows to 0 for the whole block, NOMASK if the block is exactly
    zero, else MASKED.  first_chunk counts leading fully-masked 128-wide
    q chunks (exp underflows to exactly 0 there, so they are skipped)."""
    kinds = []
    for g in range(NQG):
        row = []
        for kt in range(NKT):
            blk = maskT[kt * P:(kt + 1) * P, g * QG:(g + 1) * QG]
            if np.all(blk <= -1e5):
                row.append((SKIP, 0))
            elif not blk.any():
                row.append((NOMASK, 0))
            else:
                fc = 0
                while (fc < NCH - 1
                       and np.all(blk[:, fc * P:(fc + 1) * P] <= -1e5)):
                    fc += 1
                row.append((MASKED, fc))
        kinds.append(tuple(row))
    # every q column must have at least one contributing block, else the
    # softmax denominator would be 0 (exp-underflow trick assumption)
    for g in range(NQG):
        for j in range(NCH):
            assert any(k != SKIP and f <= j for k, f in kinds[g]), (g, j)
    return tuple(kinds)


def _get_nc(block_kind):
    key = ("nc", block_kind)
    if key not in _CACHE:
        _CACHE[key] = _build(block_kind)
    return _CACHE[key]


def _prep_inputs(x, freqs_cos, freqs_sin, mask, wq, wk, wv, wo):
    import ml_dtypes

    f = np.float32
    bf = ml_dtypes.bfloat16
    x = np.asarray(x, f).reshape(S, D)
    mask = np.asarray(mask, f).reshape(S, S)
    wq, wk, wv, wo = (np.asarray(w, f) for w in (wq, wk, wv, wo))
    cos = np.asarray(freqs_cos, f)
    sin = np.asarray(freqs_sin, f)

    xT = np.ascontiguousarray(x.T).astype(bf)
    maskT = np.ascontiguousarray(mask.T)
    woT = np.ascontiguousarray(wo.T).astype(bf)

    C = np.repeat(cos.T, 2, axis=0)          # [128, S], rows 2j,2j+1 = cos_j
    Sg = np.repeat(sin.T, 2, axis=0)
    Sg[0::2] *= -1.0                          # even rows: -sin, odd: +sin
    scale = 1.0 / math.sqrt(DH)
    common = {
        "xT": xT, "maskT": maskT.astype(bf), "woT": woT,
        "cq": np.ascontiguousarray(C * scale),
        "sq": np.ascontiguousarray(Sg * scale),
        "ck": C, "sk": Sg,
        "ones": np.ones((P, 1), bf),
    }
    in_maps = []
    for c in range(N_CORES):
        rows = slice(c * NH, (c + 1) * NH)
        in_maps.append(dict(
            common,
            wqT=np.ascontiguousarray(wq[rows].T).astype(bf),
            wkT=np.ascontiguousarray(wk[rows].T).astype(bf),
            wvT=np.ascontiguousarray(wv[rows].T).astype(bf),
        ))
    return in_maps


def kernel(x, freqs_cos, freqs_sin, mask, wq, wk, wv, wo, start_pos):
    from concourse.bass_utils import run_bass_kernel_spmd

    in_maps = _prep_inputs(x, freqs_cos, freqs_sin, mask, wq, wk, wv, wo)
    maskT_f32 = np.asarray(mask, np.float32).reshape(S, S).T
    nc = _get_nc(_classify_mask(np.ascontiguousarray(maskT_f32)))
    res = run_bass_kernel_spmd(nc, in_maps, core_ids=list(range(N_CORES)))
    full = np.concatenate([res.results[c]["out"] for c in range(N_CORES)],
                          axis=0)
    return full.reshape(1, S, D).astype(np.float32)


# revision 9
# speedup vs baseline: 1.0124x; 1.0124x over previous
"""Trainium2 Bass kernel for a 16-head dense attention block (B=1, S=2048, D=2048).

Sharding: 2 heads per core across 8 cores (tensor parallel on heads).
The reference's (deliberate) transpose(2,3)+reshape before the output
projection makes output rows [h*128:(h+1)*128) depend ONLY on head h, so
per-core outputs are disjoint row blocks -> host-side concat, no collectives.

Fully fused single-loop pipeline; per iteration g (q-group of 512):
  B: scores S^T[k,q] = K @ Q^T (f32r) for all active k-tiles, exp on ScalarE
     -> bf16 prob tiles (kept live for the whole group), PE-paced by weaving
     in A's matmuls.
  A: QKV projections for group g+1 (all-bf16 matmuls, full rate), RoPE on
     DVE with pair-swap via SBUF->SBUF DMA; V stored bf16.
  C: O[q,dh] computed directly per 128-q chunk (stationary = prob chunk,
     moving = V block, bf16): accumulation groups run j-outer/kt-inner so
     each PSUM bank hosts one contiguous group at a time.  Row-sums are
     near-free flipped matmuls (moving = ones [k,1], 1 row).  Epilogue:
     DVE reciprocal + 4 tensor_scalar_muls fold 1/rowsum into bf16 O.
  D: output projection partials R_h += O_h(g)^T @ wo^T(g-rows) streamed
     per (mg,h) through one PSUM bank ring, ScalarE PSUM->SBUF copy, and
     DMA-accumulated into DRAM out (gpsimd software-DGE adds).
Causal-mask block skipping at 128x128 chunk granularity (exp underflow).
PSUM: pq(2) + pv/sums(1) + sc(3) + pso/psr(2) = 8 banks exactly.
"""

import math

import numpy as np

S = 2048
D = 2048
H = 16
DH = 128
N_CORES = 8
HPC = H // N_CORES          # heads per core
NH = HPC * DH               # per-core head rows (256)
P = 128
QG = 512                    # q/s-group width
NQG = S // QG               # 4
NKT = S // P                # 16 k tiles
NDT = D // P                # 16 d tiles
NCH = QG // P               # 4 chunks per group

SKIP, NOMASK, MASKED = 0, 1, 2

_CACHE = {}


def _build(block_kind):
    """block_kind: tuple of NQG tuples of NKT (kind, first_chunk) pairs."""
    import os
    import concourse.tile as tile
    from concourse import bacc, mybir
    from concourse.alu_op_type import AluOpType

    B = lambda k, d: int(os.environ.get(k, d))
    f32 = mybir.dt.float32
    f32r = mybir.dt.float32r
    bf16 = mybir.dt.bfloat16
    EXP = mybir.ActivationFunctionType.Exp

    nc = bacc.Bacc("TRN2", target_bir_lowering=False, debug=False,
                   num_devices=N_CORES)

    xT = nc.dram_tensor("xT", [D, S], bf16, kind="ExternalInput").ap()
    wqT = nc.dram_tensor("wqT", [D, NH], bf16, kind="ExternalInput").ap()
    wkT = nc.dram_tensor("wkT", [D, NH], bf16, kind="ExternalInput").ap()
    wvT = nc.dram_tensor("wvT", [D, NH], bf16, kind="ExternalInput").ap()
    maskT = nc.dram_tensor("maskT", [S, S], bf16, kind="ExternalInput").ap()
    woT = nc.dram_tensor("woT", [S, D], bf16, kind="ExternalInput").ap()
    cq = nc.dram_tensor("cq", [DH, S], f32, kind="ExternalInput").ap()
    sq = nc.dram_tensor("sq", [DH, S], f32, kind="ExternalInput").ap()
    ck = nc.dram_tensor("ck", [DH, S], f32, kind="ExternalInput").ap()
    sk = nc.dram_tensor("sk", [DH, S], f32, kind="ExternalInput").ap()
    ones = nc.dram_tensor("ones", [P, 1], bf16, kind="ExternalInput").ap()
    out = nc.dram_tensor("out", [NH, D], f32, kind="ExternalOutput").ap()

    xT_v = xT.rearrange("(t p) s -> t p s", p=P)           # [16,128,S]
    wT_v = {"q": wqT.rearrange("(t p) n -> t p n", p=P),
            "k": wkT.rearrange("(t p) n -> t p n", p=P),
            "v": wvT.rearrange("(t p) n -> t p n", p=P)}
    maskT_v = maskT.rearrange("(t p) s -> t p s", p=P)
    woT_v = woT.rearrange("(t p) m -> t p m", p=P)

    # per-group mask info
    act = []          # active kts per g
    jkt = []          # per g: {j: [kts writing chunk j]}
    for g in range(NQG):
        kinds = block_kind[g]
        a = [kt for kt in range(NKT) if kinds[kt][0] != SKIP]
        act.append(a)
        jkt.append({j: [kt for kt in a if kinds[kt][1] <= j]
                    for j in range(NCH)})

    with tile.TileContext(nc) as tc:
        with tc.tile_pool(name="pers", bufs=1) as pers, \
             tc.tile_pool(name="wsb", bufs=1) as wsb, \
             tc.tile_pool(name="xssb", bufs=1) as xssb, \
             tc.tile_pool(name="ptsb", bufs=1) as ptsb, \
             tc.tile_pool(name="attsb", bufs=1) as attsb, \
             tc.tile_pool(name="wosb", bufs=B("BW", 16)) as wosb, \
             tc.tile_pool(name="rssb", bufs=2) as rssb, \
             tc.tile_pool(name="ropesb", bufs=B("BR", 2)) as ropesb, \
             tc.tile_pool(name="ps_q", bufs=B("BQ", 2), space="PSUM") as ps_q, \
             tc.tile_pool(name="ps_vs", bufs=1, space="PSUM") as ps_vs, \
             tc.tile_pool(name="ps_sc", bufs=B("BS", 3), space="PSUM") as ps_sc, \
             tc.tile_pool(name="ps_or", bufs=B("BO", 2), space="PSUM") as ps_or:

            qt = [pers.tile([P, S], f32, tag=f"qt{h}", name=f"qt{h}")
                  for h in range(HPC)]
            kt_ = [pers.tile([P, S], f32, tag=f"kt{h}", name=f"kt{h}")
                   for h in range(HPC)]
            vt = pers.tile([P, NKT, NH], bf16, tag="v")
            o_sb = [pers.tile([P, S], bf16, tag=f"ot{h}", name=f"ot{h}")
                    for h in range(HPC)]
            rt = [pers.tile([P, NKT], f32, tag=f"rt{h}", name=f"rt{h}")
                  for h in range(HPC)]

            # weights + rope constants (scalar queue); chunk-major order so
            # the first q-projection matmul's operands land first
            wts = {kind: wsb.tile([P, NDT, NH], bf16, tag=f"w{kind}",
                                  name=f"w{kind}")
                   for kind in ("q", "k", "v")}
            for c in range(4):
                cs = slice(c * 4, c * 4 + 4)
                for kind in ("q", "k", "v"):
                    nc.scalar.dma_start(
                        wts[kind][:, cs],
                        wT_v[kind][cs].rearrange("t p n -> p t n"))
            ones_t = pers.tile([P, 1], bf16, tag="ones")
            nc.scalar.dma_start(ones_t[:], ones[:])
            rope_t = {}
            for nm, src in (("cq", cq), ("sq", sq), ("ck", ck), ("sk", sk)):
                t = wsb.tile([DH, S], f32, tag=nm, name=nm)
                nc.scalar.dma_start(t[:], src[:])
                rope_t[nm] = t

            xs_tiles = {}     # g -> list of 4 chunk tiles

            def issue_xs(g):
                xs_c = []
                sl = slice(g * QG, (g + 1) * QG)
                for c in range(4):
                    cs = slice(c * 4, c * 4 + 4)
                    xc = xssb.tile([P, 4, QG], bf16, tag=f"xs{c}", bufs=2,
                                   name=f"xs{c}")
                    nc.sync.dma_start(
                        xc[:], xT_v[cs, :, sl].rearrange("t p s -> p t s"))
                    xs_c.append(xc)
                xs_tiles[g] = xs_c

            def qkv_items(g):
                """Flat list of emit-callbacks for QKV projections of group
                g: matmuls at item granularity; group-final items also emit
                the PSUM->SBUF copy (+ rope)."""
                xs_c = xs_tiles[g]
                sl = slice(g * QG, (g + 1) * QG)
                items = []

                def xs_ap(dt, *rest):
                    return xs_c[dt // 4][(slice(None), dt % 4) + rest]

                def emit_rope(kind, h, ps):
                    m = qt[h] if kind == "q" else kt_[h]
                    cn, sn = ("cq", "sq") if kind == "q" else ("ck", "sk")
                    nc.vector.tensor_copy(m[:, sl].bitcast(f32r), ps[:])
                    sw = ropesb.tile([P, QG], f32, tag="sw", name="sw")
                    m_v = m[:, sl].rearrange("(j b) s -> j b s", b=2)
                    sw_v = sw.rearrange("(j b) s -> j b s", b=2)
                    nc.scalar.dma_start(sw_v[:, 0], m_v[:, 1])
                    nc.scalar.dma_start(sw_v[:, 1], m_v[:, 0])
                    t1 = ropesb.tile([P, QG], f32, tag="t1", name="t1")
                    nc.vector.tensor_mul(t1[:], m[:, sl], rope_t[cn][:, sl])
                    nc.vector.tensor_mul(sw[:], sw[:], rope_t[sn][:, sl])
                    nc.vector.tensor_add(m[:, sl].bitcast(f32r), t1[:], sw[:])

                for kind in ("q", "k"):
                    for h in range(HPC):
                        ps = ps_q.tile([P, QG], f32, tag="pq", name="pq")
                        for dt in range(NDT):
                            def mk(kind=kind, h=h, ps=ps, dt=dt):
                                nc.tensor.matmul(
                                    ps[:], wts[kind][:, dt, h * P:(h + 1) * P],
                                    xs_ap(dt), start=(dt == 0),
                                    stop=(dt == NDT - 1))
                                if dt == NDT - 1:
                                    emit_rope(kind, h, ps)
                            items.append(mk)
                for st in range(g * 4, g * 4 + 4):
                    ps = ps_vs.tile([P, QG], f32, tag="vs", name="pvs")
                    lsl = slice((st % 4) * P, (st % 4) * P + P)
                    for dt in range(NDT):
                        def mk(st=st, ps=ps, lsl=lsl, dt=dt):
                            nc.tensor.matmul(ps[:, 0:NH], xs_ap(dt, lsl),
                                             wts["v"][:, dt], start=(dt == 0),
                                             stop=(dt == NDT - 1))
                            if dt == NDT - 1:
                                nc.vector.tensor_copy(vt[:, st], ps[:, 0:NH])
                        items.append(mk)
                return items

            # prologue: x for groups 0 and 1; QKV(0) unweaved
            issue_xs(0)
            issue_xs(1)
            for it in qkv_items(0):
                it()

            pts = {}          # (g, h, kt) -> pt tile

            def emit_mask_dma(g, kt):
                kind, fc = block_kind[g][kt]
                x_off = fc * P
                mt = attsb.tile([P, QG], bf16, tag="mt", bufs=4, name="mt")
                nc.sync.dma_start(
                    mt[:, x_off:QG],
                    maskT_v[kt][:, g * QG + x_off:(g + 1) * QG])
                return mt

            def emit_b(g, kt, mt):
                kind, fc = block_kind[g][kt]
                sc_off = min(fc * P, QG - 2 * P)
                x_off = fc * P
                osl_sc = slice(sc_off, QG)
                qsl_sc = slice(g * QG + sc_off, (g + 1) * QG)
                osl_x = slice(x_off, QG)
                ksl = slice(kt * P, (kt + 1) * P)
                for h in range(HPC):
                    ps = ps_sc.tile([P, QG], f32, tag="sc", name="sc")
                    nc.tensor.matmul(ps[:, osl_sc],
                                     kt_[h][:, ksl].bitcast(f32r),
                                     qt[h][:, qsl_sc].bitcast(f32r),
                                     start=True, stop=True)
                    pt = ptsb.tile([P, QG], bf16, tag=f"pt{h}", bufs=NKT,
                                   name=f"pt{h}")
                    if kind == MASKED:
                        sm = attsb.tile([P, QG], f32, tag="sm", bufs=2,
                                        name="sm")
                        nc.vector.tensor_add(sm[:, osl_x], ps[:, osl_x],
                                             mt[:, osl_x])
                        nc.scalar.activation(pt[:, osl_x], sm[:, osl_x], EXP)
                    else:
                        nc.scalar.activation(pt[:, osl_x], ps[:, osl_x], EXP)
                    pts[g, h, kt] = pt

            # number of g+1 k-tiles whose scores/exp are emitted early, at
            # the end of iteration g (fills ScalarE while iteration g+1 has
            # no QKV filler left)
            EARLY = {3: B("BE", 8)}

            for g in range(NQG):
                kinds = block_kind[g]
                if g + 2 < NQG:
                    issue_xs(g + 2)
                early = EARLY.get(g, 0)
                main_kts = act[g][early:]
                mts = {kt: emit_mask_dma(g, kt) for kt in main_kts
                       if kinds[kt][0] == MASKED}

                # ---- B (scores+exp) weaved with A (QKV g+1) ----
                a_items = qkv_items(g + 1) if g + 1 < NQG else []
                n_b = max(len(main_kts), 1)
                emitted = 0
                for i, kt in enumerate(main_kts):
                    emit_b(g, kt, mts.get(kt))
                    want = len(a_items) * (i + 1) // n_b
                    while emitted < want:
                        a_items[emitted]()
                        emitted += 1
                while emitted < len(a_items):
                    a_items[emitted]()
                    emitted += 1

                # ---- C: O + rowsums, j-outer/kt-inner groups ----
                pss = ps_vs.tile([P, QG], f32, tag="vs", name="pss")
                pso = {}
                for h in range(HPC):
                    pso[h] = ps_or.tile([P, QG], f32, tag="or", name="pso")
                    for j in range(NCH):
                        jsl_l = slice(j * P, (j + 1) * P)
                        ks = jkt[g][j]
                        for kt in ks:
                            nc.tensor.matmul(
                                pso[h][:, jsl_l], pts[g, h, kt][:, jsl_l],
                                vt[:, kt, h * P:(h + 1) * P],
                                start=(kt == ks[0]), stop=(kt == ks[-1]))
                        col = NH + h * NCH + j
                        for kt in ks:
                            nc.tensor.matmul(
                                pss[:, col:col + 1], pts[g, h, kt][:, jsl_l],
                                ones_t[:],
                                start=(kt == ks[0]), stop=(kt == ks[-1]))
                    c0 = NH + h * NCH
                    nc.vector.reciprocal(rt[h][:, g * NCH:(g + 1) * NCH],
                                         pss[:, c0:c0 + NCH])
                    for j in range(NCH):
                        jt = g * NCH + j
                        jsl = slice(jt * P, (jt + 1) * P)
                        nc.vector.tensor_scalar_mul(
                            o_sb[h][:, jsl], pso[h][:, j * P:(j + 1) * P],
                            rt[h][:, jt:jt + 1])

                # ---- D: output projection partials for this group ----
                for mg in range(NQG):
                    msl = slice(mg * QG, (mg + 1) * QG)
                    wt_c = []
                    for ji in range(NCH):
                        jt = g * NCH + ji
                        wt = wosb.tile([P, QG], bf16, tag="wo", name="wt")
                        nc.scalar.dma_start(wt[:], woT_v[jt][:, msl])
                        wt_c.append(wt)
                    for h in range(HPC):
                        psr = ps_or.tile([P, QG], f32, tag="or", name="psr")
                        for ji in range(NCH):
                            jt = g * NCH + ji
                            jsl = slice(jt * P, (jt + 1) * P)
                            nc.tensor.matmul(psr[:], o_sb[h][:, jsl],
                                             wt_c[ji], start=(ji == 0),
                                             stop=(ji == NCH - 1))
                        rs = rssb.tile([P, QG], f32, tag="rs", name="rs")
                        nc.scalar.copy(rs[:], psr[:])
                        if g == 0:
                            nc.sync.dma_start(out[h * P:(h + 1) * P, msl],
                                              rs[:])
                        else:
                            nc.gpsimd.dma_start(out[h * P:(h + 1) * P, msl],
                                                rs[:],
                                                accum_op=AluOpType.add)

                # ---- early scores/exp for the next group ----
                nearly = EARLY.get(g + 1, 0)
                if nearly:
                    early_kts = act[g + 1][:nearly]
                    emts = {kt: emit_mask_dma(g + 1, kt) for kt in early_kts
                            if block_kind[g + 1][kt][0] == MASKED}
                    for kt in early_kts:
                        emit_b(g + 1, kt, emts.get(kt))

    nc.compile()
    return nc


def _classify_mask(maskT):
    """Per (g, kt) block: (kind, first_chunk).  kind is SKIP if exp(s+m)
    underflows to 0 for the whole block, NOMASK if the block is exactly
    zero, else MASKED.  first_chunk counts leading fully-masked 128-wide
    q chunks (exp underflows to exactly 0 there, so they are skipped)."""
    kinds = []
    for g in range(NQG):
        row = []
        for kt in range(NKT):
            blk = maskT[kt * P:(kt + 1) * P, g * QG:(g + 1) * QG]
            if np.all(blk <= -1e5):
                row.append((SKIP, 0))
            elif not blk.any():
                row.append((NOMASK, 0))
            else:
                fc = 0
                while (fc < NCH - 1
                       and np.all(blk[:, fc * P:(fc + 1) * P] <= -1e5)):
                    fc += 1
                row.append((MASKED, fc))
        kinds.append(tuple(row))
    # every q column must have at least one contributing block, else the
    # softmax denominator would be 0 (exp-underflow trick assumption)
    for g in range(NQG):
        for j in range(NCH):
            assert any(k != SKIP and f <= j for k, f in kinds[g]), (g, j)
    return tuple(kinds)


def _get_nc(block_kind):
    key = ("nc", block_kind)
    if key not in _CACHE:
        _CACHE[key] = _build(block_kind)
    return _CACHE[key]


def _prep_inputs(x, freqs_cos, freqs_sin, mask, wq, wk, wv, wo):
    import ml_dtypes

    f = np.float32
    bf = ml_dtypes.bfloat16
    x = np.asarray(x, f).reshape(S, D)
    mask = np.asarray(mask, f).reshape(S, S)
    wq, wk, wv, wo = (np.asarray(w, f) for w in (wq, wk, wv, wo))
    cos = np.asarray(freqs_cos, f)
    sin = np.asarray(freqs_sin, f)

    xT = np.ascontiguousarray(x.T).astype(bf)
    maskT = np.ascontiguousarray(mask.T)
    woT = np.ascontiguousarray(wo.T).astype(bf)

    C = np.repeat(cos.T, 2, axis=0)          # [128, S], rows 2j,2j+1 = cos_j
    Sg = np.repeat(sin.T, 2, axis=0)
    Sg[0::2] *= -1.0                          # even rows: -sin, odd: +sin
    scale = 1.0 / math.sqrt(DH)
    common = {
        "xT": xT, "maskT": maskT.astype(bf), "woT": woT,
        "cq": np.ascontiguousarray(C * scale),
        "sq": np.ascontiguousarray(Sg * scale),
        "ck": C, "sk": Sg,
        "ones": np.ones((P, 1), bf),
    }
    in_maps = []
    for c in range(N_CORES):
        rows = slice(c * NH, (c + 1) * NH)
        in_maps.append(dict(
            common,
            wqT=np.ascontiguousarray(wq[rows].T).astype(bf),
            wkT=np.ascontiguousarray(wk[rows].T).astype(bf),
            wvT=np.ascontiguousarray(wv[rows].T).astype(bf),
        ))
    return in_maps


def kernel(x, freqs_cos, freqs_sin, mask, wq, wk, wv, wo, start_pos):
    from concourse.bass_utils import run_bass_kernel_spmd

    in_maps = _prep_inputs(x, freqs_cos, freqs_sin, mask, wq, wk, wv, wo)
    maskT_f32 = np.asarray(mask, np.float32).reshape(S, S).T
    nc = _get_nc(_classify_mask(np.ascontiguousarray(maskT_f32)))
    res = run_bass_kernel_spmd(nc, in_maps, core_ids=list(range(N_CORES)))
    full = np.concatenate([res.results[c]["out"] for c in range(N_CORES)],
                          axis=0)
    return full.reshape(1, S, D).astype(np.float32)
